# revision 1
# baseline (speedup 1.0000x reference)
"""Trainium2 Bass kernel: 2-layer CompGATv3 encoder + ConvE decoder (KG link scoring).

Sharding (8 NeuronCores, SPMD, full inputs in / full output out):
- Node-parallel GNN: core c owns entity rows [c*6250, (c+1)*6250). Host sorts
  edges by destination and buckets them into 128-node blocks; each block's
  edge list is padded to whole 128-edge tiles. The tile schedule is the
  per-block max over cores, so one program serves all cores (SPMD).
- Per edge tile: indirect-DMA gathers (source embedding, relation vector,
  destination self-term), message matmul on PE (transpose + 2 matmuls),
  GATv2 logits via Lrelu + tensor_tensor_reduce, exp without max-subtraction
  (logits are tiny by construction), scatter-add through a one-hot selection
  matmul into per-block PSUM accumulators. The segment softmax denominator
  is accumulated as an extra ones-column and divided out at the node level.
- Layer boundary: AllGather of the new entity slice (bf16).
- Decoder: conv lowered to a host-built sparse matrix, output-column-sharded
  over cores with a partial-z AllReduce; score matmul uses the core's local
  entity slice (DMA-transposed) so the [B, n_ent] output is column-sharded.
"""

import math
import numpy as np
import ml_dtypes

import concourse.bacc as bacc
import concourse.bass as bass
import concourse.mybir as mybir
import concourse.tile as tile
import concourse.bass_utils as bass_utils
from concourse.bass import IndirectOffsetOnAxis
from concourse.masks import make_identity

F32 = mybir.dt.float32
BF16 = mybir.dt.bfloat16
I32 = mybir.dt.int32
AF = mybir.ActivationFunctionType
OP = mybir.AluOpType
BF16_NP = ml_dtypes.bfloat16

FULL_CFG = dict(n_ent=50000, n_rel=500, d=200, b=256, ncores=8,
                ent_h=10, ent_w=20, fc=32, fs=3)

BETA = 0.5
BN_EPS = 1e-5
LRELU_SLOPE = 0.2
SOFTMAX_EPS = 1e-16
PAD_COL = 999.0


# ---------------------------------------------------------------- host prep

def _ceil_div(a, b):
    return -(-a // b)


def _preprocess(inputs, cfg):
    ncores = cfg["ncores"]
    n_ent, n_rel, d, b = cfg["n_ent"], cfg["n_rel"], cfg["d"], cfg["b"]
    npc = n_ent // ncores
    nblk = _ceil_div(npc, 128)
    npad = nblk * 128

    src = np.asarray(inputs["edge_index"][0], np.int64)
    dst = np.asarray(inputs["edge_index"][1], np.int64)
    et = np.asarray(inputs["edge_type"], np.int64)

    core_of = dst // npc
    cnts = np.zeros((ncores, nblk), np.int64)
    percore = []
    for c in range(ncores):
        m = core_of == c
        s_c, d_c, t_c = src[m], dst[m], et[m]
        o = np.argsort(d_c, kind="stable")
        s_c, d_c, t_c = s_c[o], d_c[o], t_c[o]
        loc = (d_c - c * npc).astype(np.int64)
        blk = loc // 128
        cnts[c] = np.bincount(blk, minlength=nblk)
        percore.append((s_c, t_c, loc, blk))

    tpb = np.maximum(1, _ceil_div(cnts.max(axis=0), 128)).astype(np.int64)
    T = int(tpb.sum())
    tile_blk = np.repeat(np.arange(nblk), tpb)
    tstart = np.zeros(nblk, np.int64)
    tstart[1:] = np.cumsum(tpb)[:-1]

    srcT = np.zeros((ncores, 128, T), np.int32)
    etT = np.zeros((ncores, 128, T), np.int32)
    dlocT = np.zeros((ncores, 128, T), np.int32)
    colT = np.full((ncores, 128, T), PAD_COL, np.float32)
    for c in range(ncores):
        s_c, t_c, loc, blk = percore[c]
        off = np.zeros(nblk, np.int64)
        off[1:] = np.cumsum(cnts[c])[:-1]
        wb = np.arange(len(s_c)) - off[blk]          # index within block
        slot = tstart[blk] * 128 + wb                # flat slot in [T*128]
        fs_ = np.zeros(T * 128, np.int32)
        ft = np.zeros(T * 128, np.int32)
        fd = np.zeros(T * 128, np.int32)
        fc_ = np.full(T * 128, PAD_COL, np.float32)
        fs_[slot] = s_c
        ft[slot] = t_c
        fd[slot] = loc
        fc_[slot] = (loc % 128).astype(np.float32)
        srcT[c] = fs_.reshape(T, 128).T
        etT[c] = ft.reshape(T, 128).T
        dlocT[c] = fd.reshape(T, 128).T
        colT[c] = fc_.reshape(T, 128).T

    f32 = lambda x: np.ascontiguousarray(np.asarray(x, np.float32))
    bf = lambda x: np.ascontiguousarray(np.asarray(x, np.float32).astype(BF16_NP))

    ent_emb = f32(inputs["ent_emb"])
    rel_emb = f32(inputs["rel_emb"])

    def aug(w, a):
        # [d, d+1]: last column is w @ a (linear part of the attention logit)
        w = f32(w)
        return np.concatenate([w, (w @ f32(a))[:, None]], axis=1)

    entT = []
    for c in range(ncores):
        sl = np.zeros((d, npad), np.float32)
        sl[:, :npc] = ent_emb[c * npc:(c + 1) * npc].T
        entT.append(bf(sl))

    # ---- decoder prep
    ent_h, ent_w, fc, fs_k = cfg["ent_h"], cfg["ent_w"], cfg["fc"], cfg["fs"]
    hh, ww = 2 * ent_h, ent_w                 # image dims (20, 20)
    oh, ow = hh - fs_k + 1, ww - fs_k + 1     # conv output (18, 18)
    num_in = fc * oh * ow
    npix = hh * ww                            # 400
    conv_w = f32(inputs["conv_w"])            # [fc, 1, fs, fs]
    g0p = float(np.asarray(inputs["bn0_g"], np.float32)[0] / math.sqrt(1.0 + BN_EPS))
    b0 = float(np.asarray(inputs["bn0_b"], np.float32)[0])
    g1p = f32(inputs["bn1_g"]) / math.sqrt(1.0 + BN_EPS)
    b1v = f32(inputs["bn1_b"])
    gpp = f32(inputs["bnp_g"]) / math.sqrt(1.0 + BN_EPS)
    bpv = f32(inputs["bnp_b"])
    prelu1 = float(np.asarray(inputs["prelu1"], np.float32).ravel()[0])
    prelu2 = float(np.asarray(inputs["prelu2"], np.float32).ravel()[0])

    big_w = np.zeros((npix, num_in), np.float32)
    oy, ox = np.meshgrid(np.arange(oh), np.arange(ow), indexing="ij")
    for oc in range(fc):
        for dy in range(fs_k):
            for dx in range(fs_k):
                pix = (oy + dy) * ww + (ox + dx)
                out_i = oc * (oh * ow) + oy * ow + ox
                big_w[pix, out_i] = conv_w[oc, 0, dy, dx] * g0p
    # pixel reorder: [head dims 0..d-1, tail dims 0..d-1] (orig interleaved 2d, 2d+1)
    perm = np.concatenate([np.arange(d) * 2, np.arange(d) * 2 + 1])
    big_w = big_w[perm]

    ocpc = num_in // ncores          # out-columns per core
    occ = fc // ncores               # conv channels per core
    sumw = conv_w.reshape(fc, -1).sum(1)
    nchunk = _ceil_div(ocpc, 128)
    acol = np.zeros((ncores, nchunk * 128, 1), np.float32)
    ccol = np.zeros((ncores, nchunk * 128, 1), np.float32)
    for c in range(ncores):
        ocs = np.arange(ocpc) // (oh * ow) + c * occ
        acol[c, :ocpc, 0] = g1p[ocs]
        ccol[c, :ocpc, 0] = g1p[ocs] * b0 * sumw[ocs] + b1v[ocs]

    acol_a = acol * prelu1           # scale/bias for the linear branch of prelu
    ccol_a = ccol * prelu1

    pw = f32(inputs["proj_w"]) * gpp[None, :]
    pb = f32(inputs["proj_b"]) * gpp + bpv
    pwc = np.zeros((ncores, ocpc + 1, d), np.float32)
    for c in range(ncores):
        pwc[c, :ocpc] = pw[c * ocpc:(c + 1) * ocpc]
    pwc[0, ocpc] = pb                      # bias row only on core 0 (AllReduce sums)

    bias_ent = f32(inputs["bias_ent"])
    bias_sl = np.zeros((ncores, 1, npad), np.float32)
    for c in range(ncores):
        bias_sl[c, 0, :npc] = bias_ent[c * npc:(c + 1) * npc]

    hidx = np.asarray(inputs["h"], np.int64).astype(np.int32)
    ridx = np.asarray(inputs["r"], np.int64).astype(np.int32)
    bb = b // 128                           # batch chunks (2)
    hidx2 = hidx.reshape(bb, 128).T.copy()  # [128, bb]
    ridx2 = ridx.reshape(bb, 128).T.copy()

    common = {
        "ent_tab": bf(ent_emb),
        "rel_tab": bf(rel_emb),
        "relT": bf(rel_emb.T),
        "W1": bf(aug(inputs["W1"], inputs["a1"])),
        "Ws1": bf(aug(inputs["Wself1"], inputs["a1"])),
        "W2": bf(aug(inputs["W2"], inputs["a2"])),
        "Ws2": bf(aug(inputs["Wself2"], inputs["a2"])),
        "Wr1": bf(inputs["Wrel1"]), "Wr2": bf(inputs["Wrel2"]),
        "A1m": f32(np.broadcast_to(np.asarray(inputs["a1"], np.float32), (128, d))),
        "A2m": f32(np.broadcast_to(np.asarray(inputs["a2"], np.float32), (128, d))),
        "B1m": f32(np.broadcast_to(np.asarray(inputs["b1"], np.float32), (128, d))),
        "B2m": f32(np.broadcast_to(np.asarray(inputs["b2"], np.float32), (128, d))),
        "hidx": hidx2, "ridx": ridx2,
    }
    per_core = []
    for c in range(ncores):
        per_core.append({
            "srcT": srcT[c], "etT": etT[c], "dlocT": dlocT[c], "colT": colT[c],
            "entT_hi": entT[c][:128], "entT_lo": entT[c][128:d],
            "bigW": bf(big_w[:, c * ocpc:(c + 1) * ocpc]),
            "acol": acol[c], "ccol": ccol[c],
            "acol_a": acol_a[c], "ccol_a": ccol_a[c],
            "pwc": bf(pwc[c]),
            "bias_sl": bf(bias_sl[c]),
        })

    sched = dict(T=T, tpb=tpb, tile_blk=tile_blk, tstart=tstart,
                 npc=npc, nblk=nblk, npad=npad, nchunk=nchunk, ocpc=ocpc,
                 prelu1=prelu1, prelu2=prelu2, npix=npix, bb=bb)
    return common, per_core, sched


# ---------------------------------------------------------------- device code

def _dchunks(d):
    out = []
    s = 0
    while s < d:
        out.append((s, min(s + 128, d)))
        s += 128
    return out


def _emit_v_phase(nc, pool, ps_pool, xT_tiles, w_dram, v_dram, nblk, npc, d):
    """v[n,:] = x[n,:] @ [Wself | Wself@a] for owned nodes; f32 to v_dram."""
    dch = _dchunks(d)
    w_sb = []
    for (s, e) in dch:
        wt = pool.tile([e - s, d + 1], BF16, tag=f"vw{s}")
        nc.sync.dma_start(wt[:], w_dram[s:e, :])
        w_sb.append(wt)
    for bkt in range(nblk):
        vps = ps_pool.tile([128, d + 1], F32, tag="ups")
        for i, (s, e) in enumerate(dch):
            nc.tensor.matmul(vps[:], lhsT=xT_tiles[i][0:e - s, bkt * 128:(bkt + 1) * 128],
                             rhs=w_sb[i][:], start=(i == 0), stop=(i == len(dch) - 1))
        vsb = pool.tile([128, d + 1], F32, tag="vsb")
        nc.vector.tensor_copy(vsb[:], vps[:])
        nc.sync.dma_start(v_dram[bkt * 128:(bkt + 1) * 128, :], vsb[:])


def _emit_rel_phase(nc, pool, ps_pool, relT_dram, wr1_dram, wr2_dram,
                    rel1_dram, rel2_dram, n_rel, d):
    """rel1 = rel @ Wr1 ; rel2 = rel1 @ Wr2 (row-major bf16 tables in DRAM)."""
    dch = _dchunks(d)
    relT_sb, wr1_sb, wr2_sb = [], [], []
    for (s, e) in dch:
        t = pool.tile([e - s, n_rel], BF16, tag=f"relT{s}")
        nc.sync.dma_start(t[:], relT_dram[s:e, :])
        relT_sb.append(t)
        t1 = pool.tile([e - s, d], BF16, tag=f"wr1{s}")
        nc.sync.dma_start(t1[:], wr1_dram[s:e, :])
        wr1_sb.append(t1)
        t2 = pool.tile([e - s, d], BF16, tag=f"wr2{s}")
        nc.sync.dma_start(t2[:], wr2_dram[s:e, :])
        wr2_sb.append(t2)
    # rel1T[do, r] = sum_di Wr1[di, do] relT[di, r]
    rel1T_sb = []
    for (s, e) in dch:
        t = pool.tile([e - s, n_rel], BF16, tag=f"rel1T{s}")
        rel1T_sb.append(t)
    for oi, (os_, oe) in enumerate(dch):
        rps = ps_pool.tile([128, n_rel], F32, tag="ups")
        for i, (s, e) in enumerate(dch):
            nc.tensor.matmul(rps[0:oe - os_, :], lhsT=wr1_sb[i][:, os_:oe],
                             rhs=relT_sb[i][:], start=(i == 0), stop=(i == len(dch) - 1))
        nc.vector.tensor_copy(rel1T_sb[oi][:], rps[0:oe - os_, :])
    # row-major rel1 / rel2 in chunks of <=128 relations
    for r0 in range(0, n_rel, 128):
        r1 = min(r0 + 128, n_rel)
        rps = ps_pool.tile([128, d], F32, tag="ups")
        for i, (s, e) in enumerate(dch):
            nc.tensor.matmul(rps[0:r1 - r0, :], lhsT=relT_sb[i][:, r0:r1],
                             rhs=wr1_sb[i][:], start=(i == 0), stop=(i == len(dch) - 1))
        rsb = pool.tile([128, d], BF16, tag="rel1sb")
        nc.vector.tensor_copy(rsb[0:r1 - r0, :], rps[0:r1 - r0, :])
        nc.sync.dma_start(rel1_dram[r0:r1, :], rsb[0:r1 - r0, :])
        rps2 = ps_pool.tile([128, d], F32, tag="ups")
        for i, (s, e) in enumerate(dch):
            nc.tensor.matmul(rps2[0:r1 - r0, :], lhsT=rel1T_sb[i][:, r0:r1],
                             rhs=wr2_sb[i][:], start=(i == 0), stop=(i == len(dch) - 1))
        rsb2 = pool.tile([128, d], BF16, tag="rel2sb")
        nc.vector.tensor_copy(rsb2[0:r1 - r0, :], rps2[0:r1 - r0, :])
        nc.sync.dma_start(rel2_dram[r0:r1, :], rsb2[0:r1 - r0, :])


def _emit_edge_layer(nc, tc, layer, cst, epool, ps_tr, ps_u, ps_acc, sched,
                     x_tab, r_tab, v_dram, w_sb, amat, bmat, idx, ident_bf,
                     iota_f32, ex_store, rd1_store, out_sinks, d):
    """One CompGAT layer over all edge tiles + per-block epilogues."""
    T, tpb, tile_blk, tstart = sched["T"], sched["tpb"], sched["tile_blk"], sched["tstart"]
    npc, nblk = sched["npc"], sched["nblk"]
    srcT_sb, etT_sb, dlocT_sb, colT_sb = idx
    dch = _dchunks(d)
    acc = None
    acc2 = None
    for t in range(T):
        bkt = int(tile_blk[t])
        j = t - int(tstart[bkt])
        last = j == int(tpb[bkt]) - 1
        xs = epool.tile([128, d], BF16, tag="xs")
        nc.gpsimd.indirect_dma_start(
            out=xs[:], out_offset=None, in_=x_tab[:, :],
            in_offset=IndirectOffsetOnAxis(ap=srcT_sb[:, t:t + 1], axis=0))
        re = epool.tile([128, d], BF16, tag="re")
        nc.gpsimd.indirect_dma_start(
            out=re[:], out_offset=None, in_=r_tab[:, :],
            in_offset=IndirectOffsetOnAxis(ap=etT_sb[:, t:t + 1], axis=0))
        vd = epool.tile([128, d + 1], F32, tag="vd")
        nc.gpsimd.indirect_dma_start(
            out=vd[:], out_offset=None, in_=v_dram[:, :],
            in_offset=IndirectOffsetOnAxis(ap=dlocT_sb[:, t:t + 1], axis=0))
        comp = epool.tile([128, d], BF16, tag="comp")
        nc.vector.tensor_tensor(out=comp[:], in0=xs[:], in1=re[:], op=OP.mult)
        trp = ps_tr.tile([128, 256], BF16, tag="trp")
        cts = []
        for i, (s, e) in enumerate(dch):
            nc.tensor.transpose(out=trp[0:e - s, i * 128:i * 128 + 128],
                                in_=comp[:, s:e], identity=ident_bf[:])
            ct = epool.tile([e - s, 128], BF16, tag=f"ct{i}")
            if i == 0:
                nc.vector.tensor_copy(ct[:], trp[0:e - s, i * 128:i * 128 + 128])
            else:
                nc.scalar.copy(ct[:], trp[0:e - s, i * 128:i * 128 + 128])
            cts.append(ct)
        ups = ps_u.tile([128, d + 1], F32, tag="ups")
        for i, (s, e) in enumerate(dch):
            nc.tensor.matmul(ups[:], lhsT=cts[i][:], rhs=w_sb[i][:],
                             start=(i == 0), stop=(i == len(dch) - 1))
        # z = msg + self-term; col d is z@a (linear logit part)
        z = epool.tile([128, d + 1], F32, tag="z")
        nc.vector.tensor_tensor(out=z[:], in0=ups[:], in1=vd[:], op=OP.add)
        # lrelu(z)@a = c1*(z@a) + c2*(|z|@a), c1=(1+s)/2, c2=(1-s)/2
        abz = epool.tile([128, d], F32, tag="abz")
        nc.scalar.activation(abz[:], z[:, 0:d], AF.Abs)
        e0 = epool.tile([128, 1], F32, tag="e0")
        nc.vector.tensor_scalar(out=e0[:], in0=z[:, d:d + 1],
                                scalar1=(1.0 + LRELU_SLOPE) / 2.0,
                                scalar2=None, op0=OP.mult)
        ttro = epool.tile([128, d], F32, tag="ttro")
        e_sb = epool.tile([128, 1], F32, tag="esb")
        nc.vector.scalar_tensor_tensor(out=ttro[:], in0=abz[:], scalar=1.0,
                                       in1=amat[:], op0=OP.mult, op1=OP.mult,
                                       accum_out=e_sb[:])
        if layer == 1:
            ex_ap = ex_store[:, t:t + 1]
        else:
            ex_t = epool.tile([128, 1], F32, tag="ex2")
            ex_ap = ex_t[:, :]
        nc.scalar.activation(ex_ap, e_sb[:], AF.Exp,
                             scale=(1.0 - LRELU_SLOPE) / 2.0, bias=e0[:, 0:1])
        uaug = epool.tile([128, d + 1], BF16, tag="uaug")
        nc.vector.tensor_copy(uaug[:, 0:d], ups[:, 0:d])
        nc.gpsimd.memset(uaug[:, d:d + 1], 1.0)
        sex = epool.tile([128, 128], BF16, tag="sex")
        nc.vector.tensor_scalar(out=sex[:], in0=iota_f32[:],
                                scalar1=colT_sb[:, t:t + 1], scalar2=ex_ap,
                                op0=OP.is_equal, op1=OP.mult)
        if j == 0:
            acc = ps_acc.tile([128, d + 1], F32, tag="acc")
        nc.tensor.matmul(acc[:], lhsT=sex[:], rhs=uaug[:], start=(j == 0), stop=last)
        if layer == 2:
            sex1 = epool.tile([128, 128], BF16, tag="sex1")
            nc.vector.tensor_scalar(out=sex1[:], in0=iota_f32[:],
                                    scalar1=colT_sb[:, t:t + 1],
                                    scalar2=ex_store[:, t:t + 1],
                                    op0=OP.is_equal, op1=OP.mult)
            if j == 0:
                acc2 = ps_acc.tile([128, d + 1], F32, tag="acc2")
            nc.tensor.matmul(acc2[:], lhsT=sex1[:], rhs=uaug[:],
                             start=(j == 0), stop=last)
        if last:
            _emit_block_epilogue(nc, layer, cst, epool, acc, acc2, bkt, sched,
                                 bmat, rd1_store, out_sinks, d)


def _emit_block_epilogue(nc, layer, cst, epool, acc, acc2, bkt, sched,
                         bmat, rd1_store, out_sinks, d):
    npc = sched["npc"]
    rows = min(128, npc - bkt * 128)
    den_eps = epool.tile([128, 1], F32, tag="deneps")
    nc.vector.tensor_scalar(out=den_eps[:], in0=acc[:, d:d + 1],
                            scalar1=SOFTMAX_EPS, scalar2=None, op0=OP.add)
    if layer == 1:
        rd_ap = rd1_store[:, bkt:bkt + 1]
        nc.vector.reciprocal(rd_ap, den_eps[:])
        t1 = epool.tile([128, d], F32, tag="ep_t1")
        nc.vector.tensor_scalar(out=t1[:], in0=acc[:, 0:d], scalar1=rd_ap,
                                scalar2=None, op0=OP.mult)
    else:
        rd2 = epool.tile([128, 1], F32, tag="rd2")
        nc.vector.reciprocal(rd2[:], den_eps[:])
        tB = epool.tile([128, d], F32, tag="ep_tB")
        nc.vector.tensor_scalar(out=tB[:], in0=acc[:, 0:d], scalar1=rd2[:, :],
                                scalar2=1.0 - BETA, op0=OP.mult, op1=OP.mult)
        tA = epool.tile([128, d], F32, tag="ep_tA")
        nc.vector.tensor_scalar(out=tA[:], in0=acc2[:, 0:d],
                                scalar1=rd1_store[:, bkt:bkt + 1],
                                scalar2=BETA, op0=OP.mult, op1=OP.mult)
        t1 = epool.tile([128, d], F32, tag="ep_t1")
        nc.vector.tensor_tensor(out=t1[:], in0=tA[:], in1=tB[:], op=OP.add)
    t2 = epool.tile([128, d], F32, tag="ep_t2")
    nc.vector.tensor_tensor(out=t2[:], in0=t1[:], in1=bmat[:], op=OP.add)
    ebf = epool.tile([128, 256], BF16, tag="ep_ebf")
    nc.scalar.activation(ebf[:, 0:d], t2[:], AF.Tanh)
    nc.gpsimd.memset(ebf[:, d:256], 0.0)
    sl_dram, pad_dram = out_sinks
    if rows > 0:
        nc.sync.dma_start(sl_dram[bkt * 128:bkt * 128 + rows, :], ebf[0:rows, 0:d])
    nc.sync.dma_start(pad_dram[bkt * 128:(bkt + 1) * 128, :], ebf[:, :])


def _emit_decoder(nc, tc, cst, pool, ps_pool, sched, tensors, d, b, n_rel, prelu1, prelu2):
    npad, nchunk, ocpc, bb = sched["npad"], sched["nchunk"], sched["ocpc"], sched["bb"]
    npc = sched["npc"]
    (ent2_full, rel2_dram, ent2_pad, bigw_dram, acol_dram, ccol_dram,
     acola_dram, ccola_dram, pwc_dram,
     bias_dram, hidx_dram, ridx_dram, z_in, z_ar, scores_out, ident_bf) = tensors
    dch = _dchunks(d)
    npix = sched["npix"]

    hidx_sb = cst.tile([128, bb], I32, tag="hidx")
    nc.sync.dma_start(hidx_sb[:], hidx_dram[:, :])
    ridx_sb = cst.tile([128, bb], I32, tag="ridx")
    nc.sync.dma_start(ridx_sb[:], ridx_dram[:, :])

    # gather + transpose head/tail-rel into imgT K-chunks [d-chunks x b]
    imgT = []
    for nm in ("h", "r"):
        for (s, e) in dch:
            t = cst.tile([e - s, bb * 128], BF16, tag=f"imgT{nm}{s}")
            imgT.append(t)
    for bc in range(bb):
        head = pool.tile([128, d], BF16, tag="dec_head")
        nc.gpsimd.indirect_dma_start(
            out=head[:], out_offset=None, in_=ent2_full[:, :],
            in_offset=IndirectOffsetOnAxis(ap=hidx_sb[:, bc:bc + 1], axis=0))
        rrep = pool.tile([128, d], BF16, tag="dec_rrep")
        nc.gpsimd.indirect_dma_start(
            out=rrep[:], out_offset=None, in_=rel2_dram[:, :],
            in_offset=IndirectOffsetOnAxis(ap=ridx_sb[:, bc:bc + 1], axis=0))
        for gi, g in enumerate((head, rrep)):
            for i, (s, e) in enumerate(dch):
                tp = ps_pool.tile([128, 128], BF16, tag="ups")
                nc.tensor.transpose(out=tp[0:e - s, 0:128], in_=g[:, s:e],
                                    identity=ident_bf[:])
                nc.scalar.copy(imgT[gi * len(dch) + i][:, bc * 128:(bc + 1) * 128],
                               tp[0:e - s, 0:128])

    # conv via big sparse matrix: K-chunks follow [head dims, tail dims] order
    bw_sb = []
    kch = []
    r0 = 0
    for nm_i in range(2):
        for (s, e) in dch:
            kch.append((r0, r0 + (e - s)))
            r0 += e - s
    for i, (s, e) in enumerate(kch):
        t = cst.tile([e - s, ocpc], BF16, tag=f"bw{i}")
        nc.sync.dma_start(t[:], bigw_dram[s:e, :])
        bw_sb.append(t)
    acol_sb = cst.tile([128, nchunk], F32, tag="acol")
    nc.sync.dma_start(acol_sb[:], acol_dram[:, :].rearrange("(c p) o -> p (c o)", p=128))
    ccol_sb = cst.tile([128, nchunk], F32, tag="ccol")
    nc.sync.dma_start(ccol_sb[:], ccol_dram[:, :].rearrange("(c p) o -> p (c o)", p=128))
    acola_sb = cst.tile([128, nchunk], F32, tag="acola")
    nc.sync.dma_start(acola_sb[:], acola_dram[:, :].rearrange("(c p) o -> p (c o)", p=128))
    ccola_sb = cst.tile([128, nchunk], F32, tag="ccola")
    nc.sync.dma_start(ccola_sb[:], ccola_dram[:, :].rearrange("(c p) o -> p (c o)", p=128))

    ones_row = cst.tile([1, bb * 128], BF16, tag="ones_row")
    nc.gpsimd.memset(ones_row[:], 1.0)

    yT = []
    for ci in range(nchunk):
        cols = min(128, ocpc - ci * 128)
        yt = cst.tile([cols, bb * 128], BF16, tag=f"yT{ci}")
        cps = ps_pool.tile([128, bb * 128], F32, tag="ups")
        for i in range(len(kch)):
            nc.tensor.matmul(cps[0:cols, :], lhsT=bw_sb[i][:, ci * 128:ci * 128 + cols],
                             rhs=imgT[i][:], start=(i == 0), stop=(i == len(kch) - 1))
        # prelu(w) = a*w + (1-a)*relu(w), w = A*conv + C
        wlin = pool.tile([128, bb * 128], F32, tag="dec_wlin")
        nc.scalar.activation(wlin[0:cols, :], cps[0:cols, :], AF.Identity,
                             scale=acola_sb[0:cols, ci:ci + 1],
                             bias=ccola_sb[0:cols, ci:ci + 1])
        wrel = pool.tile([128, bb * 128], F32, tag="dec_wrel")
        nc.scalar.activation(wrel[0:cols, :], cps[0:cols, :], AF.Relu,
                             scale=acol_sb[0:cols, ci:ci + 1],
                             bias=ccol_sb[0:cols, ci:ci + 1])
        wrs = pool.tile([128, bb * 128], F32, tag="dec_wrs")
        nc.vector.tensor_scalar(out=wrs[0:cols, :], in0=wrel[0:cols, :],
                                scalar1=1.0 - prelu1, scalar2=None, op0=OP.mult)
        nc.vector.tensor_tensor(out=yt[0:cols, :], in0=wlin[0:cols, :],
                                in1=wrs[0:cols, :], op=OP.add)
        yT.append(yt)

    # proj: z[b, d] partial = sum_ci yT_ci.T @ pw_ci  + ones.T @ pb (core 0 only)
    pbrow = cst.tile([1, d], BF16, tag="pbrow")
    nc.sync.dma_start(pbrow[:], pwc_dram[ocpc:ocpc + 1, :])
    for bc in range(bb):
        zps = ps_pool.tile([128, d], F32, tag="ups")
        for ci in range(nchunk):
            cols = min(128, ocpc - ci * 128)
            pwt = pool.tile([cols, d], BF16, tag="pwt")
            nc.sync.dma_start(pwt[:], pwc_dram[ci * 128:ci * 128 + cols, :])
            nc.tensor.matmul(zps[:], lhsT=yT[ci][:, bc * 128:(bc + 1) * 128],
                             rhs=pwt[:], start=(ci == 0), stop=False)
        nc.tensor.matmul(zps[:], lhsT=ones_row[0:1, bc * 128:(bc + 1) * 128],
                         rhs=pbrow[:], start=False, stop=True)
        zsb = pool.tile([128, d], F32, tag="dec_zsb")
        nc.vector.tensor_copy(zsb[:], zps[:])
        nc.sync.dma_start(z_in[bc * 128:(bc + 1) * 128, :], zsb[:])

    nc.gpsimd.collective_compute(
        "AllReduce", OP.add, replica_groups=[list(range(FULL_CFG["ncores"]))],
        ins=[z_in.ap()], outs=[z_ar.ap()])

    # prelu2 + transpose z2
    z2 = pool.tile([128, bb * d], F32, tag="z2")
    for bc in range(bb):
        nc.sync.dma_start(z2[:, bc * d:(bc + 1) * d], z_ar[bc * 128:(bc + 1) * 128, :])
    z2r = pool.tile([128, bb * d], F32, tag="z2r")
    nc.scalar.activation(z2r[:], z2[:], AF.Relu, scale=1.0 - prelu2)
    z2l = pool.tile([128, bb * d], F32, tag="z2l")
    nc.vector.tensor_scalar(out=z2l[:], in0=z2[:], scalar1=prelu2, scalar2=None,
                            op0=OP.mult)
    z2p = pool.tile([128, bb * d], BF16, tag="z2p")
    nc.vector.tensor_tensor(out=z2p[:], in0=z2l[:], in1=z2r[:], op=OP.add)
    z2T_hi = cst.tile([128, bb * 128], BF16, tag="z2T_hi")
    lo = d - 128
    z2T_lo = cst.tile([lo, bb * 128], BF16, tag="z2T_lo")
    for bc in range(bb):
        for i, (s, e) in enumerate(dch):
            tp = ps_pool.tile([128, 128], BF16, tag="ups")
            nc.tensor.transpose(out=tp[0:e - s, 0:128],
                                in_=z2p[:, bc * d + s:bc * d + e], identity=ident_bf[:])
            tgt = z2T_hi if i == 0 else z2T_lo
            nc.scalar.copy(tgt[0:e - s, bc * 128:(bc + 1) * 128], tp[0:e - s, 0:128])

    # ent2^T via DMA transpose (bf16); entity bias via ones-row matmul
    e2T_hi = cst.tile([128, npad], BF16, tag="e2T_hi")
    nc.sync.dma_start_transpose(e2T_hi[:], ent2_pad[:, 0:128])
    e2T_lo = cst.tile([128, npad], BF16, tag="e2T_lo")
    nc.sync.dma_start_transpose(e2T_lo[:], ent2_pad[:, 128:256])
    biasrow = cst.tile([1, npad], BF16, tag="biasrow")
    nc.sync.dma_start(biasrow[:], bias_dram[0:1, :])

    for ns in range(0, npad, 512):
        ne = min(ns + 512, npad)
        valid = min(ne, npc) - ns
        if valid <= 0:
            continue
        for bc in range(bb):
            sps = ps_pool.tile([128, ne - ns], F32, tag="ups")
            nc.tensor.matmul(sps[:], lhsT=z2T_hi[:, bc * 128:(bc + 1) * 128],
                             rhs=e2T_hi[:, ns:ne], start=True, stop=False)
            nc.tensor.matmul(sps[:], lhsT=z2T_lo[:, bc * 128:(bc + 1) * 128],
                             rhs=e2T_lo[0:lo, ns:ne], start=False, stop=False)
            nc.tensor.matmul(sps[:], lhsT=ones_row[0:1, bc * 128:(bc + 1) * 128],
                             rhs=biasrow[0:1, ns:ne], start=False, stop=True)
            ssb = pool.tile([128, ne - ns], F32, tag="dec_ssb")
            if bc % 2 == 0:
                nc.vector.tensor_copy(ssb[:], sps[:])
            else:
                nc.scalar.copy(ssb[:], sps[:])
            nc.sync.dma_start(scores_out[bc * 128:(bc + 1) * 128, ns:ns + valid],
                              ssb[:, 0:valid])


def build_program(common, per_core, sched, cfg):
    ncores, d, b, n_rel, n_ent = (cfg["ncores"], cfg["d"], cfg["b"],
                                  cfg["n_rel"], cfg["n_ent"])
    T, npc, nblk, npad = sched["T"], sched["npc"], sched["nblk"], sched["npad"]
    nchunk, ocpc, bb = sched["nchunk"], sched["ocpc"], sched["bb"]

    nc = bacc.Bacc("TRN2", target_bir_lowering=False, debug=False,
                   num_devices=ncores)

    di = {}
    def inp(name, arr_shape, dt):
        di[name] = nc.dram_tensor(name, list(arr_shape), dt, kind="ExternalInput")
        return di[name]

    inp("srcT", (128, T), I32); inp("etT", (128, T), I32)
    inp("dlocT", (128, T), I32); inp("colT", (128, T), F32)
    inp("ent_tab", (n_ent, d), BF16); inp("rel_tab", (n_rel, d), BF16)
    inp("relT", (d, n_rel), BF16)
    for w in ("W1", "Ws1", "W2", "Ws2"):
        inp(w, (d, d + 1), BF16)
    for w in ("Wr1", "Wr2"):
        inp(w, (d, d), BF16)
    for w in ("A1m", "A2m", "B1m", "B2m"):
        inp(w, (128, d), F32)
    inp("entT_hi", (128, npad), BF16); inp("entT_lo", (d - 128, npad), BF16)
    inp("bigW", (2 * d, ocpc), BF16)
    inp("acol", (nchunk * 128, 1), F32); inp("ccol", (nchunk * 128, 1), F32)
    inp("acol_a", (nchunk * 128, 1), F32); inp("ccol_a", (nchunk * 128, 1), F32)
    inp("pwc", (ocpc + 1, d), BF16)
    inp("bias_sl", (1, npad), BF16)
    inp("hidx", (128, bb), I32); inp("ridx", (128, bb), I32)

    scores_out = nc.dram_tensor("scores", [b, npc], F32, kind="ExternalOutput")

    # internal DRAM
    v1_dram = nc.dram_tensor("v1_dram", [npad, d + 1], F32, kind="Internal")
    v2_dram = nc.dram_tensor("v2_dram", [npad, d + 1], F32, kind="Internal")
    ent1_sl = nc.dram_tensor("ent1_sl", [npc, d], BF16, kind="Internal")
    ent2_sl = nc.dram_tensor("ent2_sl", [npc, d], BF16, kind="Internal")
    ent1_pad = nc.dram_tensor("ent1_pad", [npad, 256], BF16, kind="Internal")
    ent2_pad = nc.dram_tensor("ent2_pad", [npad, 256], BF16, kind="Internal")
    rel1_dram = nc.dram_tensor("rel1_dram", [n_rel, d], BF16, kind="Internal")
    rel2_dram = nc.dram_tensor("rel2_dram", [n_rel, d], BF16, kind="Internal")
    z_in = nc.dram_tensor("z_in", [b, d], F32, kind="Internal")
    ent1_full = nc.dram_tensor("ent1_full", [n_ent, d], BF16, kind="Internal",
                               addr_space="Shared")
    ent2_full = nc.dram_tensor("ent2_full", [n_ent, d], BF16, kind="Internal",
                               addr_space="Shared")
    z_ar = nc.dram_tensor("z_ar", [b, d], F32, kind="Internal",
                          addr_space="Shared")

    dch = _dchunks(d)
    with tile.TileContext(nc) as tc:
        with tc.tile_pool(name="cst", bufs=1) as cst, \
             tc.tile_pool(name="epool", bufs=3) as epool, \
             tc.tile_pool(name="vpool", bufs=2) as vpool, \
             tc.tile_pool(name="ps_tr", bufs=2, space="PSUM") as ps_tr, \
             tc.tile_pool(name="ps_u", bufs=2, space="PSUM") as ps_u, \
             tc.tile_pool(name="ps_acc", bufs=2, space="PSUM") as ps_acc:

            ident_bf = cst.tile([128, 128], BF16, tag="ident_bf")
            make_identity(nc, ident_bf[:])
            iota_i = cst.tile([128, 128], I32, tag="iota_i")
            nc.gpsimd.iota(iota_i[:], pattern=[[1, 128]], base=0, channel_multiplier=0)
            iota_f32 = cst.tile([128, 128], F32, tag="iota_f32")
            nc.vector.tensor_copy(iota_f32[:], iota_i[:])

            idx = []
            for nm, dt in (("srcT", I32), ("etT", I32), ("dlocT", I32), ("colT", F32)):
                t = cst.tile([128, T], dt, tag=f"idx_{nm}")
                nc.sync.dma_start(t[:], di[nm][:, :])
                idx.append(t)

            mats = {}
            for nm in ("A1m", "A2m", "B1m", "B2m"):
                t = cst.tile([128, d], F32, tag=nm)
                nc.sync.dma_start(t[:], di[nm][:, :])
                mats[nm] = t
            w_sb = {}
            for nm in ("W1", "W2"):
                w_sb[nm] = []
                for (s, e) in dch:
                    t = cst.tile([e - s, d + 1], BF16, tag=f"{nm}_{s}")
                    nc.sync.dma_start(t[:], di[nm][s:e, :])
                    w_sb[nm].append(t)

            ex_store = cst.tile([128, T], F32, tag="ex_store")
            rd1_store = cst.tile([128, nblk], F32, tag="rd1_store")

            # v1 from host-transposed ent slice
            entT_tiles = []
            for i, (s, e) in enumerate(dch):
                t = cst.tile([e - s, npad], BF16, tag=f"entT{i}")
                nc.sync.dma_start(t[:], di["entT_hi" if i == 0 else "entT_lo"][:, :])
                entT_tiles.append(t)
            _emit_v_phase(nc, vpool, ps_u, entT_tiles, di["Ws1"], v1_dram,
                          nblk, npc, d)
            _emit_rel_phase(nc, vpool, ps_u, di["relT"], di["Wr1"], di["Wr2"],
                            rel1_dram, rel2_dram, n_rel, d)

            # ---- layer 1
            _emit_edge_layer(nc, tc, 1, cst, epool, ps_tr, ps_u, ps_acc, sched,
                             di["ent_tab"], di["rel_tab"], v1_dram, w_sb["W1"],
                             mats["A1m"], mats["B1m"], idx, ident_bf, iota_f32,
                             ex_store, rd1_store, (ent1_sl, ent1_pad), d)

            nc.gpsimd.collective_compute(
                "AllGather", OP.bypass, replica_groups=[list(range(ncores))],
                ins=[ent1_sl.ap()], outs=[ent1_full.ap()])

            # v2 from DMA-transposed ent1
            e1T_tiles = []
            for i in range(2):
                t = cst.tile([128, npad], BF16, tag=f"e1T{i}")
                nc.sync.dma_start_transpose(t[:], ent1_pad[:, i * 128:(i + 1) * 128])
                e1T_tiles.append(t)
            _emit_v_phase(nc, vpool, ps_u, e1T_tiles, di["Ws2"], v2_dram,
                          nblk, npc, d)

            # ---- layer 2
            _emit_edge_layer(nc, tc, 2, cst, epool, ps_tr, ps_u, ps_acc, sched,
                             ent1_full, rel1_dram, v2_dram, w_sb["W2"],
                             mats["A2m"], mats["B2m"], idx, ident_bf, iota_f32,
                             ex_store, rd1_store, (ent2_sl, ent2_pad), d)

            nc.gpsimd.collective_compute(
                "AllGather", OP.bypass, replica_groups=[list(range(ncores))],
                ins=[ent2_sl.ap()], outs=[ent2_full.ap()])

            # ---- decoder
            _emit_decoder(nc, tc, cst, vpool, ps_u, sched,
                          (ent2_full, rel2_dram, ent2_pad, di["bigW"], di["acol"],
                           di["ccol"], di["acol_a"], di["ccol_a"],
                           di["pwc"], di["bias_sl"], di["hidx"],
                           di["ridx"], z_in, z_ar, scores_out, ident_bf),
                          d, b, n_rel, sched["prelu1"], sched["prelu2"])

    nc.compile()
    return nc


# ---------------------------------------------------------------- entry

_CACHE = {}


def _run(inputs, cfg, sim=False, trace=False):
    common, per_core, sched = _preprocess(inputs, cfg)
    key = (tuple(sorted(cfg.items())), sched["T"], tuple(sched["tpb"]))
    if key not in _CACHE:
        _CACHE[key] = build_program(common, per_core, sched, cfg)
    nc = _CACHE[key]
    in_maps = []
    for c in range(cfg["ncores"]):
        m = dict(common)
        m.update(per_core[c])
        in_maps.append({k: np.ascontiguousarray(v) for k, v in m.items()})
    if sim:
        from concourse.bass_interp import MultiCoreSim
        ms = MultiCoreSim(nc, num_cores=cfg["ncores"])
        for c in range(cfg["ncores"]):
            for name, arr in in_maps[c].items():
                ms.cores[c].tensor(name)[:] = arr
        ms.simulate(check_with_hw=False)
        outs = [np.array(ms.cores[c].tensor("scores")) for c in range(cfg["ncores"])]
        return np.concatenate(outs, axis=1), None
    res = bass_utils.run_bass_kernel_spmd(
        nc, in_maps, core_ids=list(range(cfg["ncores"])), trace=trace)
    outs = [res.results[c]["scores"] for c in range(cfg["ncores"])]
    return np.concatenate(outs, axis=1).astype(np.float32), res


def kernel(**inputs):
    out, _ = _run(inputs, FULL_CFG)
    return out

